# revision 1
# baseline (speedup 1.0000x reference)
"""Multi-head attention (Keras-style, relu-activated dense projections)
for Trainium2, SPMD across 8 NeuronCores.

Problem (full shapes):
    B, S, D, H = 4, 1024, 1024, 16 ; DH = 64
    qp = relu(q @ Wq + bq); kp = relu(k @ Wk + bk); vp = relu(v @ Wv + bv)
    per head h: scores = qh @ kh^T / 8 ; attn = softmax(scores)
    out = relu(concat_h(attn @ vh) @ Wo + bo)

Sharding: core c = (batch b = c//2, head-group g = c%2). Each core computes
the 8 heads of group g for batch b end-to-end and produces the partial
output projection  attn_out_g @ Wo[g*512:(g+1)*512, :]  (no bias / relu).
Host sums the two partials per batch, adds bo, applies relu.

Per-core dataflow (head pair hp = heads 2hp / 2hp+1):
  - host feeds q[b].T etc so projections contract d on the partition dim.
  - Q/K projections transposed: qpT/kpT [128, 4(hp), 1024(s)]; head 2hp at
    partitions 0:64, head 2hp+1 at 64:128 -> the K=64 score matmuls of a
    pair auto-land in different PE row groups and run concurrently.
  - scores pair writes one [128, 1024] 2-bank PSUM tile; one wide exp (ACT)
    emits ex [128, ut, 1024] bf16 (head A cols 0:512, B 512:1024).
  - attn@v: column-paired bf16 matmuls into nt[0:64] / nt[64:128].
  - softmax denominator: DVE tree-sums ex over ut, two K=128 matmuls with a
    ones column reduce partitions -> Z_A (psum row 0) / Z_B (row 32); a
    masked K=33 matmul broadcasts both to [128, 512]; wide DVE reciprocal +
    a single [128, 512] multiply writes attn_out.
  - output projection: full K=128 accumulating matmuls over head pairs.
  - matmuls in float32r (fp22, full PE rate) except the bf16 attention core.
"""

import numpy as np
from contextlib import ExitStack

import concourse.bass as bass
import concourse.mybir as mybir
import concourse.tile as tile
from concourse import bacc

# ---- constants (hardcoded per the contract; kernel.py must be self-contained)
B, S, D, H = 4, 1024, 1024, 16
DG = 512          # feature slice per core (8 heads)
HL = 8            # heads per core
DH = 64
P = 128
NCORES = 8
NJT = DG // P     # 4 feature tiles == head pairs
NST = S // P      # 8 sequence tiles
NDT = D // P      # 8 contraction tiles for projections
NPC = S // 512    # 2 query chunks of 512

F32 = mybir.dt.float32
F32R = mybir.dt.float32r
BF16 = mybir.dt.bfloat16
AF = mybir.ActivationFunctionType


def _d(ap):
    """View a float32 DRAM AP as float32r so DMAs into f32r tiles type-check.
    (walrus requires fp32r matmul operands to be *produced* as fp32r.)"""
    return ap.bitcast(F32R)


def build_bass():
    nc = bacc.Bacc("TRN2", target_bir_lowering=False, debug=False,
                   num_devices=NCORES)

    xqT = nc.dram_tensor("xqT", [D, S], F32, kind="ExternalInput").ap()
    xkT = nc.dram_tensor("xkT", [D, S], F32, kind="ExternalInput").ap()
    xvT = nc.dram_tensor("xvT", [D, S], F32, kind="ExternalInput").ap()
    wq = nc.dram_tensor("wq", [D, DG], F32, kind="ExternalInput").ap()
    wk = nc.dram_tensor("wk", [D, DG], F32, kind="ExternalInput").ap()
    wv = nc.dram_tensor("wv", [D, DG], F32, kind="ExternalInput").ap()
    bq = nc.dram_tensor("bq", [1, DG], F32, kind="ExternalInput").ap()
    bk = nc.dram_tensor("bk", [1, DG], F32, kind="ExternalInput").ap()
    bv = nc.dram_tensor("bv", [1, DG], F32, kind="ExternalInput").ap()
    wo = nc.dram_tensor("wo", [DG, D], F32, kind="ExternalInput").ap()
    ones_in = nc.dram_tensor("ones", [1, 512], F32, kind="ExternalInput").ap()
    bcm_in = nc.dram_tensor("bcmask", [33, P], F32, kind="ExternalInput").ap()
    out = nc.dram_tensor("out", [S, D], F32, kind="ExternalOutput").ap()

    with tile.TileContext(nc) as tc, ExitStack() as ctx, \
            nc.allow_low_precision(reason="fp32r/bf16 compute is intentional"):
        consts = ctx.enter_context(tc.tile_pool(name="consts", bufs=1))
        xpool = ctx.enter_context(tc.tile_pool(name="xpool", bufs=20))
        wpool = ctx.enter_context(tc.tile_pool(name="wpool", bufs=16))
        qkpool = ctx.enter_context(tc.tile_pool(name="qkpool", bufs=1))
        vpool = ctx.enter_context(tc.tile_pool(name="vpool", bufs=1))
        epool = ctx.enter_context(tc.tile_pool(name="epool", bufs=2))
        aopool = ctx.enter_context(tc.tile_pool(name="aopool", bufs=1))
        t1pool = ctx.enter_context(tc.tile_pool(name="t1pool", bufs=1))
        espool = ctx.enter_context(tc.tile_pool(name="espool", bufs=2))
        rpool = ctx.enter_context(tc.tile_pool(name="rpool", bufs=2))
        outpool = ctx.enter_context(tc.tile_pool(name="outpool", bufs=3))

        psA = ctx.enter_context(tc.tile_pool(name="psA", bufs=2, space="PSUM"))
        psB = ctx.enter_context(tc.tile_pool(name="psB", bufs=2, space="PSUM"))
        psZ = ctx.enter_context(tc.tile_pool(name="psZ", bufs=1, space="PSUM"))
        psD = ctx.enter_context(tc.tile_pool(name="psD", bufs=1, space="PSUM"))

        # --- constants
        ones = consts.tile([P, 512], F32R, tag="ones")
        nc.sync.dma_start(out=ones, in_=_d(ones_in.to_broadcast([P, 512])))
        onescol = consts.tile([P, 1], BF16, tag="onescol")
        nc.vector.memset(onescol, 1.0)
        bcmask = consts.tile([33, P], F32R, tag="bcmask")
        nc.sync.dma_start(out=bcmask, in_=_d(bcm_in))
        # zsb: persistent Z staging rows (0 and 32); fill once with finite
        # values so the masked K=33 broadcast matmul never reads NaNs.
        zsb = consts.tile([33, 512], F32R, tag="zsb")
        nc.sync.dma_start(out=zsb, in_=_d(ones_in.to_broadcast([33, 512])))

        bv_sb = consts.tile([1, DG], F32R, tag="bv")
        nc.sync.dma_start(out=bv_sb, in_=_d(bv))

        # --- transposed projections for Q and K
        qpT = qkpool.tile([P, NJT, S], F32R, tag="qpT")
        kpT = qkpool.tile([P, NJT, S], F32R, tag="kpT")

        # per-partition bias for the transposed projections (ACT bias input)
        bqT = consts.tile([P, NJT], F32, tag="bqT")
        nc.sync.dma_start(out=bqT, in_=bq[0, :].rearrange("(jt p) -> p jt", p=P))
        bkT = consts.tile([P, NJT], F32, tag="bkT")
        nc.sync.dma_start(out=bkT, in_=bk[0, :].rearrange("(jt p) -> p jt", p=P))

        def load_halves(xT, w):
            xmap = {}
            for pc in range(NPC):
                for dt_ in range(NDT):
                    xt = xpool.tile([P, 512], F32R, tag="xT")
                    nc.sync.dma_start(
                        out=xt,
                        in_=_d(xT[dt_ * P:(dt_ + 1) * P,
                                  pc * 512:(pc + 1) * 512]))
                    xmap[(dt_, pc)] = xt
            wts = []
            for dt_ in range(NDT):
                wt = wpool.tile([P, DG], F32R, tag="w")
                nc.sync.dma_start(out=wt, in_=_d(w[dt_ * P:(dt_ + 1) * P, :]))
                wts.append(wt)
            return xmap, wts

        for name, xT, w, bT, dst in (("q", xqT, wq, bqT, qpT),
                                     ("k", xkT, wk, bkT, kpT)):
            xmap, wts = load_halves(xT, w)
            for pc in range(NPC):
                for jt in range(NJT):
                    ps = psA.tile([P, 1024], F32, tag="ps")
                    half = ps[:, 0:512]
                    for dt_ in range(NDT):
                        nc.tensor.matmul(
                            half,
                            lhsT=wts[dt_][:, jt * P:(jt + 1) * P],
                            rhs=xmap[(dt_, pc)],
                            start=(dt_ == 0), stop=(dt_ == NDT - 1))
                    nc.scalar.activation(
                        dst[:, jt, pc * 512:(pc + 1) * 512], half, AF.Relu,
                        bias=bT[:, jt:jt + 1])

        # --- V projection, natural layout -> vpa [128, st, 512] bf16
        vpa = vpool.tile([P, NST, DG], BF16, tag="vpa")
        xmap, wts = load_halves(xvT, wv)
        for st in range(NST):
            ps = psA.tile([P, 1024], F32, tag="ps")
            half = ps[:, 0:512]
            for dt_ in range(NDT):
                nc.tensor.matmul(
                    half,
                    lhsT=xmap[(dt_, st // 4)][:, (st % 4) * P:(st % 4 + 1) * P],
                    rhs=wts[dt_],
                    start=(dt_ == 0), stop=False)
            nc.tensor.matmul(
                half, lhsT=ones[0:1, 0:P], rhs=bv_sb,
                start=False, stop=True)
            nc.scalar.activation(vpa[:, st, :], half, AF.Relu)

        # --- attention, one head pair x one 512-query chunk at a time.
        # pc outer: all head pairs of a query chunk finish together, so the
        # matching half of the output projection can start while the second
        # chunk's attention is still running.
        aoT3 = aopool.tile([P, NJT, S], F32R, tag="aoT3")

        # Wo by head pair (emitted here so its DMA runs during attention)
        wo3 = consts.tile([P, NJT, D], F32R, tag="wo3")
        for hp in range(NJT):
            nc.sync.dma_start(out=wo3[:, hp, :],
                              in_=_d(wo[hp * P:(hp + 1) * P, :]))

        for pc in range(NPC):
            pslice = slice(pc * 512, (pc + 1) * 512)
            for hp in range(NJT):
                hA, hB = 2 * hp, 2 * hp + 1
                ex = epool.tile([P, NST, 1024], BF16, tag="exp")
                for ut in range(NST):
                    uslice = slice(ut * P, (ut + 1) * P)
                    pw = psA.tile([P, 1024], F32, tag="ps")
                    nc.tensor.matmul(
                        pw[:, 0:512],
                        lhsT=kpT[0:DH, hp, uslice],
                        rhs=qpT[0:DH, hp, pslice],
                        start=True, stop=True)
                    nc.tensor.matmul(
                        pw[:, 512:1024],
                        lhsT=kpT[DH:P, hp, uslice],
                        rhs=qpT[DH:P, hp, pslice],
                        start=True, stop=True)
                    nc.scalar.activation(ex[:, ut, :], pw, AF.Exp, scale=0.125)
                # Z tree-sum over ut on DVE (overlaps the attn@v matmuls)
                t1 = t1pool.tile([P, 4, 1024], BF16, tag="t1")
                nc.vector.tensor_add(t1, ex[:, 0:4, :], ex[:, 4:8, :])
                nc.vector.tensor_add(t1[:, 0:2, :], t1[:, 0:2, :],
                                     t1[:, 2:4, :])
                exsum = espool.tile([P, 1024], BF16, tag="exsum")
                nc.vector.tensor_add(exsum, t1[:, 0, :], t1[:, 1, :])
                # Z_A -> psum row 0, Z_B -> psum row 32 (col group 1), then
                # stage into SBUF; emitted before attn@v so the copies are
                # long done when PE reaches the broadcast matmul.
                zps = psZ.tile([P, 512], F32, tag="z")
                nc.tensor.matmul(zps[0:1, :], lhsT=onescol,
                                 rhs=exsum[:, 0:512], start=True, stop=True)
                nc.tensor.matmul(zps[32:33, :], lhsT=onescol,
                                 rhs=exsum[:, 512:1024], start=True, stop=True)
                nc.vector.tensor_copy(zsb[0:1, :], zps[0:1, :])
                nc.vector.tensor_copy(zsb[32:33, :], zps[32:33, :])
                # attn @ v: column-paired accumulation over key tiles
                nt = psB.tile([P, 512], F32, tag="nt")
                for ut in range(NST):
                    nc.tensor.matmul(
                        nt[0:DH, :],
                        lhsT=vpa[:, ut, hA * DH:(hA + 1) * DH],
                        rhs=ex[:, ut, 0:512],
                        start=(ut == 0), stop=(ut == NST - 1),
                        skip_group_check=True)
                    nc.tensor.matmul(
                        nt[DH:P, :],
                        lhsT=vpa[:, ut, hB * DH:(hB + 1) * DH],
                        rhs=ex[:, ut, 512:1024],
                        start=(ut == 0), stop=(ut == NST - 1),
                        skip_group_check=True)
                # broadcast: rows 0:64 <- Z_A, rows 64:128 <- Z_B
                zbc = psZ.tile([P, 512], F32, tag="z")
                nc.tensor.matmul(zbc, lhsT=bcmask, rhs=zsb,
                                 start=True, stop=True)
                rcp = rpool.tile([P, 512], F32, tag="rcp")
                nc.vector.reciprocal_approx_fast(rcp, zbc)
                nc.vector.tensor_mul(aoT3[:, hp, pslice], nt, rcp)

            # output projection for this query chunk (pt = pc*4 .. pc*4+3)
            for pt in range(pc * 4, pc * 4 + 4):
                for jj in range(2):
                    po_ = psD.tile([P, 512], F32, tag="po")
                    for hp in range(NJT):
                        nc.tensor.matmul(
                            po_,
                            lhsT=aoT3[:, hp, pt * P:(pt + 1) * P],
                            rhs=wo3[:, hp, jj * 512:(jj + 1) * 512],
                            start=(hp == 0), stop=(hp == NJT - 1))
                    os_ = outpool.tile([P, 512], F32, tag="os")
                    nc.vector.tensor_copy(os_, po_)
                    nc.sync.dma_start(
                        out=out[pt * P:(pt + 1) * P, jj * 512:(jj + 1) * 512],
                        in_=os_)

    nc.compile()
    return nc


_CACHE = {}


def get_nc():
    if "nc" not in _CACHE:
        _CACHE["nc"] = build_bass()
    return _CACHE["nc"]


def make_bcmask():
    m = np.zeros((33, P), np.float32)
    m[0, 0:DH] = 1.0
    m[32, DH:P] = 1.0
    return m


def make_in_maps(q, k, v, Wq, bq, Wk, bk, Wv, bv, Wo, bo):
    q = np.asarray(q, np.float32)
    k = np.asarray(k, np.float32)
    v = np.asarray(v, np.float32)
    Wq = np.asarray(Wq, np.float32)
    Wk = np.asarray(Wk, np.float32)
    Wv = np.asarray(Wv, np.float32)
    Wo = np.asarray(Wo, np.float32)
    bq = np.asarray(bq, np.float32)
    bk = np.asarray(bk, np.float32)
    bv = np.asarray(bv, np.float32)

    qT = [np.ascontiguousarray(q[b].T) for b in range(B)]
    kT = [np.ascontiguousarray(k[b].T) for b in range(B)]
    vT = [np.ascontiguousarray(v[b].T) for b in range(B)]
    bcm = make_bcmask()

    in_maps = []
    for c in range(NCORES):
        b, g = divmod(c, 2)
        sl = slice(g * DG, (g + 1) * DG)
        in_maps.append({
            "xqT": qT[b],
            "xkT": kT[b],
            "xvT": vT[b],
            "wq": np.ascontiguousarray(Wq[:, sl]),
            "wk": np.ascontiguousarray(Wk[:, sl]),
            "wv": np.ascontiguousarray(Wv[:, sl]),
            "bq": np.ascontiguousarray(bq[sl]).reshape(1, DG),
            "bk": np.ascontiguousarray(bk[sl]).reshape(1, DG),
            "bv": np.ascontiguousarray(bv[sl]).reshape(1, DG),
            "wo": np.ascontiguousarray(Wo[sl, :]),
            "ones": np.ones((1, 512), np.float32),
            "bcmask": bcm,
        })
    return in_maps


def combine_outputs(parts, bo):
    bo = np.asarray(bo, np.float32)
    out = np.empty((B, S, D), np.float32)
    for b in range(B):
        out[b] = np.maximum(parts[2 * b] + parts[2 * b + 1] + bo[None, :], 0.0)
    return out


def run(in_maps, trace=False, **kwargs):
    from concourse.bass_utils import run_bass_kernel_spmd
    nc = get_nc()
    return run_bass_kernel_spmd(nc, in_maps, list(range(NCORES)),
                                trace=trace, **kwargs)


def kernel(q, k, v, Wq, bq, Wk, bk, Wv, bv, Wo, bo):
    in_maps = make_in_maps(q, k, v, Wq, bq, Wk, bk, Wv, bv, Wo, bo)
    res = run(in_maps)
    parts = [res.results[c]["out"] for c in range(NCORES)]
    return combine_outputs(parts, bo)



# revision 7
# speedup vs baseline: 1.3143x; 1.3143x over previous
"""Multi-head attention (Keras-style, relu-activated dense projections)
for Trainium2, SPMD across 8 NeuronCores.

Problem (full shapes):
    B, S, D, H = 4, 1024, 1024, 16 ; DH = 64
    qp = relu(q @ Wq + bq); kp = relu(k @ Wk + bk); vp = relu(v @ Wv + bv)
    per head h: scores = qh @ kh^T / 8 ; attn = softmax(scores)
    out = relu(concat_h(attn @ vh) @ Wo + bo)

Sharding: core c = (batch b = c//2, head-group g = c%2). Each core computes
the 8 heads of group g for batch b end-to-end and produces the partial
output projection  attn_out_g @ Wo[g*512:(g+1)*512, :]  (no bias / relu).
Host sums the two partials per batch, adds bo, applies relu.

v2 redesign vs the 241us baseline (which ran the PE half-clocked most of
the kernel because attention serialized scores->exp->attnV per head pair):
  - all matmul operands bf16 (halves DMA bytes, enables FWL weight loads);
    PSUM accumulation stays f32.
  - emission software-pipelines the whole kernel into the ACT exp windows:
    while exp(iter i) runs, PE does the next jt's Q/K projection, V
    projection slices, attnV of earlier iters and the first output
    projection chunk. PE never idles >3.4us -> HAM stays warm.
  - softmax denominator: DVE tree-sum over key tiles, then TWO masked
    ones-matmuls broadcast Z_A/Z_B straight into a [128,512] psum tile
    (replaces onescol reduce + staging copies + K=33 broadcast matmul).
  - projection relu+bias moved off ACT onto DVE (tensor_scalar add+max),
    so ACT runs exp back-to-back.
  - DMA: few big rearranged transfers, weights issued from GpSimd queue,
    activations from Sync, ordered so the first score matmul can start
    ~6us in.
"""

import numpy as np
import ml_dtypes
from contextlib import ExitStack

import concourse.bass as bass
import concourse.mybir as mybir
import concourse.tile as tile
from concourse import bacc

B, S, D, H = 4, 1024, 1024, 16
DG = 512          # feature slice per core (8 heads)
HL = 8            # heads per core
DH = 64
P = 128
NCORES = 8
NJT = DG // P     # 4 feature tiles == head pairs
NST = S // P      # 8 sequence tiles
NDT = D // P      # 8 contraction tiles for projections
NPC = S // 512    # 2 query chunks of 512

F32 = mybir.dt.float32
BF16 = mybir.dt.bfloat16
AF = mybir.ActivationFunctionType
ALU = mybir.AluOpType
BFNP = ml_dtypes.bfloat16


def build_bass():
    nc = bacc.Bacc("TRN2", target_bir_lowering=False, debug=False,
                   num_devices=NCORES)

    xqT = nc.dram_tensor("xqT", [D, S], BF16, kind="ExternalInput").ap()
    xkT = nc.dram_tensor("xkT", [D, S], BF16, kind="ExternalInput").ap()
    xvT = nc.dram_tensor("xvT", [D, S], BF16, kind="ExternalInput").ap()
    wq = nc.dram_tensor("wq", [D, DG], BF16, kind="ExternalInput").ap()
    wk = nc.dram_tensor("wk", [D, DG], BF16, kind="ExternalInput").ap()
    wv = nc.dram_tensor("wv", [D, DG], BF16, kind="ExternalInput").ap()
    bq = nc.dram_tensor("bq", [1, DG], F32, kind="ExternalInput").ap()
    bk = nc.dram_tensor("bk", [1, DG], F32, kind="ExternalInput").ap()
    bv = nc.dram_tensor("bv", [1, DG], BF16, kind="ExternalInput").ap()
    wo = nc.dram_tensor("wo", [DG, D], BF16, kind="ExternalInput").ap()
    bcm_in = nc.dram_tensor("bcmask", [P, 2 * P], BF16,
                            kind="ExternalInput").ap()
    out = nc.dram_tensor("out", [S, D], F32, kind="ExternalOutput").ap()

    with tile.TileContext(nc) as tc, ExitStack() as ctx, \
            nc.allow_low_precision(reason="bf16 compute is intentional"):
        consts = ctx.enter_context(tc.tile_pool(name="consts", bufs=1))
        xpool = ctx.enter_context(tc.tile_pool(name="xpool", bufs=5))
        wpool = ctx.enter_context(tc.tile_pool(name="wpool", bufs=3))
        wopool = ctx.enter_context(tc.tile_pool(name="wopool", bufs=1))
        qkpool = ctx.enter_context(tc.tile_pool(name="qkpool", bufs=1))
        vpool = ctx.enter_context(tc.tile_pool(name="vpool", bufs=1))
        epool = ctx.enter_context(tc.tile_pool(name="epool", bufs=4))
        aopool = ctx.enter_context(tc.tile_pool(name="aopool", bufs=1))
        t1pool = ctx.enter_context(tc.tile_pool(name="t1pool", bufs=1))
        espool = ctx.enter_context(tc.tile_pool(name="espool", bufs=2))
        rpool = ctx.enter_context(tc.tile_pool(name="rpool", bufs=4))
        outpool = ctx.enter_context(tc.tile_pool(name="outpool", bufs=2))

        psA = ctx.enter_context(tc.tile_pool(name="psA", bufs=2, space="PSUM"))
        psP = ctx.enter_context(tc.tile_pool(name="psP", bufs=1, space="PSUM"))
        psB = ctx.enter_context(tc.tile_pool(name="psB", bufs=1, space="PSUM"))
        psZ = ctx.enter_context(tc.tile_pool(name="psZ", bufs=1, space="PSUM"))
        psD = ctx.enter_context(tc.tile_pool(name="psD", bufs=1, space="PSUM"))

        # --- constants (issued on gpsimd queue; sync is reserved for x)
        bqT = consts.tile([P, NJT], F32, tag="bqT")
        nc.gpsimd.dma_start(out=bqT, in_=bq[0, :].rearrange("(jt p) -> p jt",
                                                            p=P))
        bkT = consts.tile([P, NJT], F32, tag="bkT")
        nc.gpsimd.dma_start(out=bkT, in_=bk[0, :].rearrange("(jt p) -> p jt",
                                                            p=P))
        bcm = consts.tile([P, 2 * P], BF16, tag="bcm")
        nc.gpsimd.dma_start(out=bcm, in_=bcm_in)
        bv_sb = consts.tile([1, DG], BF16, tag="bv")
        nc.gpsimd.dma_start(out=bv_sb, in_=bv)
        onesrow = consts.tile([1, P], BF16, tag="ones")
        nc.vector.memset(onesrow, 1.0)

        # --- weight tiles [128(din within dt), dt, dout] -------------------
        wkt = wpool.tile([P, NDT, DG], BF16, tag="w")
        nc.gpsimd.dma_start(out=wkt, in_=wk.rearrange("(dt p) n -> p dt n",
                                                      p=P))
        # x tiles [128(din within dt), dt, 512(seq half)]
        xk = []
        for half in range(2):
            t = xpool.tile([P, NDT, 512], BF16, tag="x")
            nc.sync.dma_start(
                out=t, in_=xkT[:, half * 512:(half + 1) * 512].rearrange(
                    "(dt p) s -> p dt s", p=P))
            xk.append(t)
        wqt = wpool.tile([P, NDT, DG], BF16, tag="w")
        nc.gpsimd.dma_start(out=wqt, in_=wq.rearrange("(dt p) n -> p dt n",
                                                      p=P))

        def load_x(src, half):
            t = xpool.tile([P, NDT, 512], BF16, tag="x")
            nc.sync.dma_start(
                out=t, in_=src[:, half * 512:(half + 1) * 512].rearrange(
                    "(dt p) s -> p dt s", p=P))
            return t

        # xq half1 is loaded later (it reuses an xk slot freed after the
        # last k-projection; q-proj of the pc1 half is deferred to W4+).
        xq = [load_x(xqT, 0), None]
        xv = [load_x(xvT, 0), load_x(xvT, 1)]
        wvt = wpool.tile([P, NDT, DG], BF16, tag="w")
        nc.gpsimd.dma_start(out=wvt, in_=wv.rearrange("(dt p) n -> p dt n",
                                                      p=P))
        wo3 = wopool.tile([P, NJT, D], BF16, tag="wo3")
        nc.gpsimd.dma_start(out=wo3, in_=wo.rearrange("(hp p) d -> p hp d",
                                                      p=P))

        # --- persistent activations ---------------------------------------
        qpT = qkpool.tile([P, NJT, S], BF16, tag="qpT")
        kpT = qkpool.tile([P, NJT, S], BF16, tag="kpT")
        vpa = vpool.tile([P, NST, DG], BF16, tag="vpa")
        aoT3 = aopool.tile([P, NJT, S], BF16, tag="aoT3")

        # ------------------------------------------------------------------
        def emit_qkproj(wt, xs, bT, dst, jt, half):
            """dst[:, jt, half*512:] = relu(w[:,jt-cols].T @ x[half] + b)"""
            ps = psP.tile([P, 512], F32, tag="pp")
            for dt_ in range(NDT):
                nc.tensor.matmul(
                    ps, lhsT=wt[:, dt_, jt * P:(jt + 1) * P],
                    rhs=xs[half][:, dt_, :],
                    start=(dt_ == 0), stop=(dt_ == NDT - 1))
            nc.vector.tensor_scalar(
                dst[:, jt, half * 512:(half + 1) * 512], ps,
                scalar1=bT[:, jt:jt + 1], scalar2=0.0,
                op0=ALU.add, op1=ALU.max)

        def emit_vproj(st):
            """vpa[:, st, :] = relu(x_v[st-cols].T @ wv + bv)"""
            ps = psP.tile([P, 512], F32, tag="pp")
            half, q = st // 4, st % 4
            for dt_ in range(NDT):
                nc.tensor.matmul(
                    ps, lhsT=xv[half][:, dt_, q * P:(q + 1) * P],
                    rhs=wvt[:, dt_, :],
                    start=(dt_ == 0), stop=False)
            nc.tensor.matmul(ps, lhsT=onesrow, rhs=bv_sb,
                             start=False, stop=True)
            nc.vector.tensor_scalar_max(vpa[:, st, :], ps, 0.0)

        def emit_scores_exp(pc, hp):
            """returns ex [128(k), ut, 1024] bf16 (head A cols 0:512, B 512:)"""
            ex = epool.tile([P, NST, 1024], BF16, tag="exp")
            pslice = slice(pc * 512, (pc + 1) * 512)
            for ut in range(NST):
                uslice = slice(ut * P, (ut + 1) * P)
                pw = psA.tile([P, 1024], F32, tag="ps")
                nc.tensor.matmul(
                    pw[:, 0:512],
                    lhsT=kpT[0:DH, hp, uslice], rhs=qpT[0:DH, hp, pslice],
                    start=True, stop=True)
                nc.tensor.matmul(
                    pw[:, 512:1024],
                    lhsT=kpT[DH:P, hp, uslice], rhs=qpT[DH:P, hp, pslice],
                    start=True, stop=True)
                nc.scalar.activation(ex[:, ut, :], pw, AF.Exp, scale=0.125)
            return ex

        def emit_finz(ex):
            """softmax denominators: rcp [128,512] f32, rows 0:64 = 1/Z_A
            broadcast, rows 64:128 = 1/Z_B."""
            t1 = t1pool.tile([P, 4, 1024], BF16, tag="t1")
            nc.vector.tensor_add(t1, ex[:, 0:4, :], ex[:, 4:8, :])
            nc.vector.tensor_add(t1[:, 0:2, :], t1[:, 0:2, :], t1[:, 2:4, :])
            exsum = espool.tile([P, 1024], BF16, tag="exsum")
            nc.vector.tensor_add(exsum, t1[:, 0, :], t1[:, 1, :])
            zps = psZ.tile([P, 512], F32, tag="z")
            nc.tensor.matmul(zps, lhsT=bcm[:, 0:P], rhs=exsum[:, 0:512],
                             start=True, stop=False)
            nc.tensor.matmul(zps, lhsT=bcm[:, P:2 * P], rhs=exsum[:, 512:1024],
                             start=False, stop=True)
            rcp = rpool.tile([P, 512], F32, tag="rcp")
            nc.vector.reciprocal_approx_fast(rcp, zps)
            return rcp

        def emit_attnv(pc, hp, ex, rcp):
            hA, hB = 2 * hp, 2 * hp + 1
            nt = psB.tile([P, 512], F32, tag="nt")
            for ut in range(NST):
                nc.tensor.matmul(
                    nt[0:DH, :],
                    lhsT=vpa[:, ut, hA * DH:(hA + 1) * DH],
                    rhs=ex[:, ut, 0:512],
                    start=(ut == 0), stop=(ut == NST - 1),
                    skip_group_check=True)
                nc.tensor.matmul(
                    nt[DH:P, :],
                    lhsT=vpa[:, ut, hB * DH:(hB + 1) * DH],
                    rhs=ex[:, ut, 512:1024],
                    start=(ut == 0), stop=(ut == NST - 1),
                    skip_group_check=True)
            nc.vector.tensor_mul(aoT3[:, hp, pc * 512:(pc + 1) * 512], nt, rcp)

        def emit_outproj(pt, copy_eng):
            os_ = outpool.tile([P, 1024], F32, tag="os")
            for jj in range(2):
                po_ = psD.tile([P, 512], F32, tag="po")
                for hp in range(NJT):
                    nc.tensor.matmul(
                        po_, lhsT=aoT3[:, hp, pt * P:(pt + 1) * P],
                        rhs=wo3[:, hp, jj * 512:(jj + 1) * 512],
                        start=(hp == 0), stop=(hp == NJT - 1))
                if copy_eng == "scalar":
                    nc.scalar.copy(os_[:, jj * 512:(jj + 1) * 512], po_)
                else:
                    nc.vector.tensor_copy(os_[:, jj * 512:(jj + 1) * 512], po_)
            nc.sync.dma_start(out=out[pt * P:(pt + 1) * P, :], in_=os_)

        # --- software-pipelined emission ----------------------------------
        # Scores for pc-chunk 0 only need the pc0 half of qpT, so pc1-half
        # q-projections are deferred to W4+; each window's PE work is sized
        # to fit under one 8-exp ACT block (~9.2us).
        # W0: minimum work before the first exp can start.
        emit_qkproj(wkt, xk, bkT, kpT, 0, 0)
        emit_qkproj(wkt, xk, bkT, kpT, 0, 1)
        emit_qkproj(wqt, xq, bqT, qpT, 0, 0)
        ex00 = emit_scores_exp(0, 0)
        # W1 (under exp(0,0))
        emit_qkproj(wkt, xk, bkT, kpT, 1, 0)
        emit_qkproj(wkt, xk, bkT, kpT, 1, 1)
        emit_qkproj(wqt, xq, bqT, qpT, 1, 0)
        emit_vproj(0)
        emit_vproj(1)
        r00 = emit_finz(ex00)
        ex01 = emit_scores_exp(0, 1)
        # W2 (under exp(0,1))
        emit_qkproj(wkt, xk, bkT, kpT, 2, 0)
        emit_qkproj(wkt, xk, bkT, kpT, 2, 1)
        emit_qkproj(wqt, xq, bqT, qpT, 2, 0)
        emit_vproj(2)
        emit_vproj(3)
        r01 = emit_finz(ex01)
        ex02 = emit_scores_exp(0, 2)
        # W3 (under exp(0,2)): last k-proj frees two x slots -> xq half1.
        emit_qkproj(wkt, xk, bkT, kpT, 3, 0)
        emit_qkproj(wkt, xk, bkT, kpT, 3, 1)
        xq[1] = load_x(xqT, 1)
        emit_qkproj(wqt, xq, bqT, qpT, 3, 0)
        emit_vproj(4)
        emit_vproj(5)
        r02 = emit_finz(ex02)
        ex03 = emit_scores_exp(0, 3)
        # W4 (under exp(0,3)): finish V proj, first attnV, start pc1 q-proj.
        emit_vproj(6)
        emit_vproj(7)
        emit_qkproj(wqt, xq, bqT, qpT, 0, 1)
        emit_attnv(0, 0, ex00, r00)
        r03 = emit_finz(ex03)
        ex10 = emit_scores_exp(1, 0)
        # W5 (under exp(1,0))
        emit_attnv(0, 1, ex01, r01)
        emit_attnv(0, 2, ex02, r02)
        emit_qkproj(wqt, xq, bqT, qpT, 1, 1)
        r10 = emit_finz(ex10)
        ex11 = emit_scores_exp(1, 1)
        # W6 (under exp(1,1))
        emit_attnv(0, 3, ex03, r03)
        emit_qkproj(wqt, xq, bqT, qpT, 2, 1)
        emit_outproj(0, "vector")
        emit_outproj(1, "vector")
        r11 = emit_finz(ex11)
        ex12 = emit_scores_exp(1, 2)
        # W7 (under exp(1,2))
        emit_attnv(1, 0, ex10, r10)
        emit_qkproj(wqt, xq, bqT, qpT, 3, 1)
        emit_outproj(2, "vector")
        emit_outproj(3, "vector")
        r12 = emit_finz(ex12)
        ex13 = emit_scores_exp(1, 3)
        # W8 (under exp(1,3))
        emit_attnv(1, 1, ex11, r11)
        emit_attnv(1, 2, ex12, r12)
        r13 = emit_finz(ex13)
        # tail
        emit_attnv(1, 3, ex13, r13)
        for pt in range(4, 8):
            emit_outproj(pt, "scalar")

    nc.compile()
    return nc


_CACHE = {}


def get_nc():
    if "nc" not in _CACHE:
        _CACHE["nc"] = build_bass()
    return _CACHE["nc"]


def make_bcmask():
    m = np.zeros((P, 2 * P), np.float32)
    m[:, 0:DH] = 1.0          # bcmA: out rows 0:64  <- Z_A
    m[:, P + DH:2 * P] = 1.0  # bcmB: out rows 64:128 <- Z_B
    return m.astype(BFNP)


def make_in_maps(q, k, v, Wq, bq, Wk, bk, Wv, bv, Wo, bo):
    q = np.asarray(q, np.float32)
    k = np.asarray(k, np.float32)
    v = np.asarray(v, np.float32)
    Wq = np.asarray(Wq, np.float32)
    Wk = np.asarray(Wk, np.float32)
    Wv = np.asarray(Wv, np.float32)
    Wo = np.asarray(Wo, np.float32)
    bq = np.asarray(bq, np.float32)
    bk = np.asarray(bk, np.float32)
    bv = np.asarray(bv, np.float32)

    qT = [np.ascontiguousarray(q[b].T).astype(BFNP) for b in range(B)]
    kT = [np.ascontiguousarray(k[b].T).astype(BFNP) for b in range(B)]
    vT = [np.ascontiguousarray(v[b].T).astype(BFNP) for b in range(B)]
    bcm = make_bcmask()

    in_maps = []
    for c in range(NCORES):
        b, g = divmod(c, 2)
        sl = slice(g * DG, (g + 1) * DG)
        in_maps.append({
            "xqT": qT[b],
            "xkT": kT[b],
            "xvT": vT[b],
            "wq": np.ascontiguousarray(Wq[:, sl]).astype(BFNP),
            "wk": np.ascontiguousarray(Wk[:, sl]).astype(BFNP),
            "wv": np.ascontiguousarray(Wv[:, sl]).astype(BFNP),
            "bq": np.ascontiguousarray(bq[sl]).reshape(1, DG),
            "bk": np.ascontiguousarray(bk[sl]).reshape(1, DG),
            "bv": np.ascontiguousarray(bv[sl]).reshape(1, DG).astype(BFNP),
            "wo": np.ascontiguousarray(Wo[sl, :]).astype(BFNP),
            "bcmask": bcm,
        })
    return in_maps


def combine_outputs(parts, bo):
    bo = np.asarray(bo, np.float32)
    out = np.empty((B, S, D), np.float32)
    for b in range(B):
        out[b] = np.maximum(parts[2 * b] + parts[2 * b + 1] + bo[None, :], 0.0)
    return out


def run(in_maps, trace=False, **kwargs):
    from concourse.bass_utils import run_bass_kernel_spmd
    nc = get_nc()
    return run_bass_kernel_spmd(nc, in_maps, list(range(NCORES)),
                                trace=trace, **kwargs)


def kernel(q, k, v, Wq, bq, Wk, bk, Wv, bv, Wo, bo):
    in_maps = make_in_maps(q, k, v, Wq, bq, Wk, bk, Wv, bv, Wo, bo)
    res = run(in_maps)
    parts = [res.results[c]["out"] for c in range(NCORES)]
    return combine_outputs(parts, bo)


# revision 12
# speedup vs baseline: 1.3754x; 1.0465x over previous
"""Multi-head attention (Keras-style, relu-activated dense projections)
for Trainium2, SPMD across 8 NeuronCores.

Problem (full shapes):
    B, S, D, H = 4, 1024, 1024, 16 ; DH = 64
    qp = relu(q @ Wq + bq); kp = relu(k @ Wk + bk); vp = relu(v @ Wv + bv)
    per head h: scores = qh @ kh^T / 8 ; attn = softmax(scores)
    out = relu(concat_h(attn @ vh) @ Wo + bo)

Sharding: core c = (batch b = c//2, head-group g = c%2). Each core computes
the 8 heads of group g for batch b end-to-end and produces the partial
output projection  attn_out_g @ Wo[g*512:(g+1)*512, :]  (no bias / relu).
Host sums the two partials per batch, adds bo, applies relu.

v2 redesign vs the 241us baseline (which ran the PE half-clocked most of
the kernel because attention serialized scores->exp->attnV per head pair):
  - all matmul operands bf16 (halves DMA bytes, enables FWL weight loads);
    PSUM accumulation stays f32.
  - emission software-pipelines the whole kernel into the ACT exp windows:
    while exp(iter i) runs, PE does the next jt's Q/K projection, V
    projection slices, attnV of earlier iters and the first output
    projection chunk. PE never idles >3.4us -> HAM stays warm.
  - softmax denominator: DVE tree-sum over key tiles, then TWO masked
    ones-matmuls broadcast Z_A/Z_B straight into a [128,512] psum tile
    (replaces onescol reduce + staging copies + K=33 broadcast matmul).
  - projection relu+bias moved off ACT onto DVE (tensor_scalar add+max),
    so ACT runs exp back-to-back.
  - DMA: few big rearranged transfers, weights issued from GpSimd queue,
    activations from Sync, ordered so the first score matmul can start
    ~6us in.
"""

import numpy as np
import ml_dtypes
from contextlib import ExitStack

import concourse.bass as bass
import concourse.mybir as mybir
import concourse.tile as tile
from concourse import bacc

B, S, D, H = 4, 1024, 1024, 16
DG = 512          # feature slice per core (8 heads)
HL = 8            # heads per core
DH = 64
P = 128
NCORES = 8
NJT = DG // P     # 4 feature tiles == head pairs
NST = S // P      # 8 sequence tiles
NDT = D // P      # 8 contraction tiles for projections
NPC = S // 512    # 2 query chunks of 512

F32 = mybir.dt.float32
BF16 = mybir.dt.bfloat16
AF = mybir.ActivationFunctionType
ALU = mybir.AluOpType
BFNP = ml_dtypes.bfloat16


def build_bass():
    nc = bacc.Bacc("TRN2", target_bir_lowering=False, debug=False,
                   num_devices=NCORES)

    # x/w tensors arrive host-pre-arranged in SBUF layout: [128(partition),
    # half, dt, 512] resp. [128, dt, dout] — every DMA line is one
    # contiguous per-partition run (128 descriptors, cheap issue).
    xqT = nc.dram_tensor("xqT", [P, 2, NDT, 512], BF16,
                         kind="ExternalInput").ap()
    xkT = nc.dram_tensor("xkT", [P, 2, NDT, 512], BF16,
                         kind="ExternalInput").ap()
    xvT = nc.dram_tensor("xvT", [P, 2, NDT, 512], BF16,
                         kind="ExternalInput").ap()
    wq = nc.dram_tensor("wq", [P, NDT, DG], BF16, kind="ExternalInput").ap()
    wk = nc.dram_tensor("wk", [P, NDT, DG], BF16, kind="ExternalInput").ap()
    wv = nc.dram_tensor("wv", [P, NDT, DG], BF16, kind="ExternalInput").ap()
    bq = nc.dram_tensor("bq", [1, DG], F32, kind="ExternalInput").ap()
    bk = nc.dram_tensor("bk", [1, DG], F32, kind="ExternalInput").ap()
    bv = nc.dram_tensor("bv", [1, DG], BF16, kind="ExternalInput").ap()
    wo = nc.dram_tensor("wo", [P, NJT, D], BF16, kind="ExternalInput").ap()
    bcm_in = nc.dram_tensor("bcmask", [P, 2 * P], BF16,
                            kind="ExternalInput").ap()
    out = nc.dram_tensor("out", [S, D], F32, kind="ExternalOutput").ap()

    with tile.TileContext(nc) as tc, ExitStack() as ctx, \
            nc.allow_low_precision(reason="bf16 compute is intentional"):
        consts = ctx.enter_context(tc.tile_pool(name="consts", bufs=1))
        xpool = ctx.enter_context(tc.tile_pool(name="xpool", bufs=5))
        wpool = ctx.enter_context(tc.tile_pool(name="wpool", bufs=3))
        wopool = ctx.enter_context(tc.tile_pool(name="wopool", bufs=1))
        qkpool = ctx.enter_context(tc.tile_pool(name="qkpool", bufs=1))
        vpool = ctx.enter_context(tc.tile_pool(name="vpool", bufs=1))
        epool = ctx.enter_context(tc.tile_pool(name="epool", bufs=4))
        aopool = ctx.enter_context(tc.tile_pool(name="aopool", bufs=1))
        t1pool = ctx.enter_context(tc.tile_pool(name="t1pool", bufs=1))
        espool = ctx.enter_context(tc.tile_pool(name="espool", bufs=2))
        rpool = ctx.enter_context(tc.tile_pool(name="rpool", bufs=4))
        outpool = ctx.enter_context(tc.tile_pool(name="outpool", bufs=2))

        psA = ctx.enter_context(tc.tile_pool(name="psA", bufs=2, space="PSUM"))
        psP = ctx.enter_context(tc.tile_pool(name="psP", bufs=1, space="PSUM"))
        psB = ctx.enter_context(tc.tile_pool(name="psB", bufs=1, space="PSUM"))
        psZ = ctx.enter_context(tc.tile_pool(name="psZ", bufs=1, space="PSUM"))
        psD = ctx.enter_context(tc.tile_pool(name="psD", bufs=1, space="PSUM"))

        # --- constants (issued on gpsimd queue; sync is reserved for x)
        bqT = consts.tile([P, NJT], F32, tag="bqT")
        nc.gpsimd.dma_start(out=bqT, in_=bq[0, :].rearrange("(jt p) -> p jt",
                                                            p=P))
        bkT = consts.tile([P, NJT], F32, tag="bkT")
        nc.gpsimd.dma_start(out=bkT, in_=bk[0, :].rearrange("(jt p) -> p jt",
                                                            p=P))
        bcm = consts.tile([P, 2 * P], BF16, tag="bcm")
        nc.gpsimd.dma_start(out=bcm, in_=bcm_in)
        bv_sb = consts.tile([1, DG], BF16, tag="bv")
        nc.gpsimd.dma_start(out=bv_sb, in_=bv)
        onesrow = consts.tile([1, P], BF16, tag="ones")
        nc.vector.memset(onesrow, 1.0)

        # --- weight tiles [128(din within dt), dt, dout] -------------------
        wkt = wpool.tile([P, NDT, DG], BF16, tag="w")
        nc.gpsimd.dma_start(out=wkt, in_=wk)
        # x tiles [128(din within dt), dt, 512(seq half)]
        xk = []
        for half in range(2):
            t = xpool.tile([P, NDT, 512], BF16, tag="x")
            nc.sync.dma_start(out=t, in_=xkT[:, half])
            xk.append(t)
        wqt = wpool.tile([P, NDT, DG], BF16, tag="w")
        nc.gpsimd.dma_start(out=wqt, in_=wq)

        def load_x(src, half):
            t = xpool.tile([P, NDT, 512], BF16, tag="x")
            nc.sync.dma_start(out=t, in_=src[:, half])
            return t

        # xq half1 is loaded later (it reuses an xk slot freed after the
        # last k-projection; q-proj of the pc1 half is deferred to W4+).
        xq = [load_x(xqT, 0), None]
        xv = [load_x(xvT, 0), load_x(xvT, 1)]
        wvt = wpool.tile([P, NDT, DG], BF16, tag="w")
        nc.gpsimd.dma_start(out=wvt, in_=wv)
        wo3 = wopool.tile([P, NJT, D], BF16, tag="wo3")
        nc.gpsimd.dma_start(out=wo3, in_=wo)

        # --- persistent activations ---------------------------------------
        qpT = qkpool.tile([P, NJT, S], BF16, tag="qpT")
        kpT = qkpool.tile([P, NJT, S], BF16, tag="kpT")
        vpa = vpool.tile([P, NST, DG], BF16, tag="vpa")
        aoT3 = aopool.tile([P, NJT, S], BF16, tag="aoT3")

        # ------------------------------------------------------------------
        def emit_qkproj(wt, xs, bT, dst, jt, half):
            """dst[:, jt, half*512:] = relu(w[:,jt-cols].T @ x[half] + b)"""
            ps = psP.tile([P, 512], F32, tag="pp")
            for dt_ in range(NDT):
                nc.tensor.matmul(
                    ps, lhsT=wt[:, dt_, jt * P:(jt + 1) * P],
                    rhs=xs[half][:, dt_, :],
                    start=(dt_ == 0), stop=(dt_ == NDT - 1))
            nc.vector.tensor_scalar(
                dst[:, jt, half * 512:(half + 1) * 512], ps,
                scalar1=bT[:, jt:jt + 1], scalar2=0.0,
                op0=ALU.add, op1=ALU.max)

        def emit_vproj(st):
            """vpa[:, st, :] = relu(x_v[st-cols].T @ wv + bv)"""
            ps = psP.tile([P, 512], F32, tag="pp")
            half, q = st // 4, st % 4
            for dt_ in range(NDT):
                nc.tensor.matmul(
                    ps, lhsT=xv[half][:, dt_, q * P:(q + 1) * P],
                    rhs=wvt[:, dt_, :],
                    start=(dt_ == 0), stop=False)
            nc.tensor.matmul(ps, lhsT=onesrow, rhs=bv_sb,
                             start=False, stop=True)
            nc.vector.tensor_scalar_max(vpa[:, st, :], ps, 0.0)

        def emit_scores_exp(pc, hp):
            """returns ex [128(k), ut, 1024] bf16 (head A cols 0:512, B 512:)"""
            ex = epool.tile([P, NST, 1024], BF16, tag="exp")
            pslice = slice(pc * 512, (pc + 1) * 512)
            for ut in range(NST):
                uslice = slice(ut * P, (ut + 1) * P)
                pw = psA.tile([P, 1024], F32, tag="ps")
                nc.tensor.matmul(
                    pw[:, 0:512],
                    lhsT=kpT[0:DH, hp, uslice], rhs=qpT[0:DH, hp, pslice],
                    start=True, stop=True)
                nc.tensor.matmul(
                    pw[:, 512:1024],
                    lhsT=kpT[DH:P, hp, uslice], rhs=qpT[DH:P, hp, pslice],
                    start=True, stop=True)
                nc.scalar.activation(ex[:, ut, :], pw, AF.Exp, scale=0.125)
            return ex

        def emit_finz(ex):
            """softmax denominators: rcp [128,512] f32, rows 0:64 = 1/Z_A
            broadcast, rows 64:128 = 1/Z_B."""
            t1 = t1pool.tile([P, 4, 1024], BF16, tag="t1")
            nc.vector.tensor_add(t1, ex[:, 0:4, :], ex[:, 4:8, :])
            nc.vector.tensor_add(t1[:, 0:2, :], t1[:, 0:2, :], t1[:, 2:4, :])
            exsum = espool.tile([P, 1024], BF16, tag="exsum")
            nc.vector.tensor_add(exsum, t1[:, 0, :], t1[:, 1, :])
            zps = psZ.tile([P, 512], F32, tag="z")
            nc.tensor.matmul(zps, lhsT=bcm[:, 0:P], rhs=exsum[:, 0:512],
                             start=True, stop=False)
            nc.tensor.matmul(zps, lhsT=bcm[:, P:2 * P], rhs=exsum[:, 512:1024],
                             start=False, stop=True)
            rcp = rpool.tile([P, 512], F32, tag="rcp")
            nc.vector.reciprocal_approx_fast(rcp, zps)
            return rcp

        def emit_attnv(pc, hp, ex, rcp):
            hA, hB = 2 * hp, 2 * hp + 1
            nt = psB.tile([P, 512], F32, tag="nt")
            for ut in range(NST):
                nc.tensor.matmul(
                    nt[0:DH, :],
                    lhsT=vpa[:, ut, hA * DH:(hA + 1) * DH],
                    rhs=ex[:, ut, 0:512],
                    start=(ut == 0), stop=(ut == NST - 1),
                    skip_group_check=True)
                nc.tensor.matmul(
                    nt[DH:P, :],
                    lhsT=vpa[:, ut, hB * DH:(hB + 1) * DH],
                    rhs=ex[:, ut, 512:1024],
                    start=(ut == 0), stop=(ut == NST - 1),
                    skip_group_check=True)
            nc.vector.tensor_mul(aoT3[:, hp, pc * 512:(pc + 1) * 512], nt, rcp)

        def emit_outproj(pt, copy_eng, alt_bank=False):
            os_ = outpool.tile([P, 1024], F32, tag="os")
            for jj in range(2):
                # after the last finz, psZ's bank is free: alternate with
                # psD so tail output-projection groups double-buffer.
                pool = psZ if (alt_bank and jj == 1) else psD
                po_ = pool.tile([P, 512], F32, tag="z" if pool is psZ
                                else "po")
                for hp in range(NJT):
                    nc.tensor.matmul(
                        po_, lhsT=aoT3[:, hp, pt * P:(pt + 1) * P],
                        rhs=wo3[:, hp, jj * 512:(jj + 1) * 512],
                        start=(hp == 0), stop=(hp == NJT - 1))
                if copy_eng == "scalar":
                    nc.scalar.copy(os_[:, jj * 512:(jj + 1) * 512], po_)
                else:
                    nc.vector.tensor_copy(os_[:, jj * 512:(jj + 1) * 512], po_)
            nc.sync.dma_start(out=out[pt * P:(pt + 1) * P, :], in_=os_)

        # --- software-pipelined emission ----------------------------------
        # Scores for pc-chunk 0 only need the pc0 half of qpT, so pc1-half
        # q-projections are deferred to W4+; each window's PE work is sized
        # to fit under one 8-exp ACT block (~9.2us).
        # W0: minimum work before the first exp can start.
        emit_qkproj(wkt, xk, bkT, kpT, 0, 0)
        emit_qkproj(wkt, xk, bkT, kpT, 0, 1)
        emit_qkproj(wqt, xq, bqT, qpT, 0, 0)
        ex00 = emit_scores_exp(0, 0)
        # W1 (under exp(0,0))
        emit_qkproj(wkt, xk, bkT, kpT, 1, 0)
        emit_qkproj(wkt, xk, bkT, kpT, 1, 1)
        emit_qkproj(wqt, xq, bqT, qpT, 1, 0)
        emit_vproj(0)
        emit_vproj(1)
        r00 = emit_finz(ex00)
        ex01 = emit_scores_exp(0, 1)
        # W2 (under exp(0,1))
        emit_qkproj(wkt, xk, bkT, kpT, 2, 0)
        emit_qkproj(wkt, xk, bkT, kpT, 2, 1)
        emit_qkproj(wqt, xq, bqT, qpT, 2, 0)
        emit_vproj(2)
        emit_vproj(3)
        r01 = emit_finz(ex01)
        ex02 = emit_scores_exp(0, 2)
        # W3 (under exp(0,2)): last k-proj frees two x slots -> xq half1.
        emit_qkproj(wkt, xk, bkT, kpT, 3, 0)
        emit_qkproj(wkt, xk, bkT, kpT, 3, 1)
        xq[1] = load_x(xqT, 1)
        emit_qkproj(wqt, xq, bqT, qpT, 3, 0)
        emit_vproj(4)
        emit_vproj(5)
        r02 = emit_finz(ex02)
        ex03 = emit_scores_exp(0, 3)
        # W4 (under exp(0,3)): finish V proj, first attnV, start pc1 q-proj.
        emit_vproj(6)
        emit_vproj(7)
        emit_qkproj(wqt, xq, bqT, qpT, 0, 1)
        emit_attnv(0, 0, ex00, r00)
        r03 = emit_finz(ex03)
        ex10 = emit_scores_exp(1, 0)
        # W5 (under exp(1,0))
        emit_attnv(0, 1, ex01, r01)
        emit_attnv(0, 2, ex02, r02)
        emit_qkproj(wqt, xq, bqT, qpT, 1, 1)
        r10 = emit_finz(ex10)
        ex11 = emit_scores_exp(1, 1)
        # W6 (under exp(1,1))
        emit_attnv(0, 3, ex03, r03)
        emit_qkproj(wqt, xq, bqT, qpT, 2, 1)
        emit_outproj(0, "vector")
        emit_outproj(1, "vector")
        r11 = emit_finz(ex11)
        ex12 = emit_scores_exp(1, 2)
        # W7 (under exp(1,2))
        emit_attnv(1, 0, ex10, r10)
        emit_qkproj(wqt, xq, bqT, qpT, 3, 1)
        emit_outproj(2, "vector")
        emit_outproj(3, "vector")
        r12 = emit_finz(ex12)
        ex13 = emit_scores_exp(1, 3)
        # W8 (under exp(1,3))
        emit_attnv(1, 1, ex11, r11)
        emit_attnv(1, 2, ex12, r12)
        r13 = emit_finz(ex13)
        # tail
        emit_attnv(1, 3, ex13, r13)
        for pt in range(4, 8):
            emit_outproj(pt, "scalar", alt_bank=True)

    nc.compile()
    return nc


_CACHE = {}


def get_nc():
    if "nc" not in _CACHE:
        _CACHE["nc"] = build_bass()
    return _CACHE["nc"]


def make_bcmask():
    m = np.zeros((P, 2 * P), np.float32)
    m[:, 0:DH] = 1.0          # bcmA: out rows 0:64  <- Z_A
    m[:, P + DH:2 * P] = 1.0  # bcmB: out rows 64:128 <- Z_B
    return m.astype(BFNP)


def make_in_maps(q, k, v, Wq, bq, Wk, bk, Wv, bv, Wo, bo):
    q = np.asarray(q, np.float32)
    k = np.asarray(k, np.float32)
    v = np.asarray(v, np.float32)
    Wq = np.asarray(Wq, np.float32)
    Wk = np.asarray(Wk, np.float32)
    Wv = np.asarray(Wv, np.float32)
    Wo = np.asarray(Wo, np.float32)
    bq = np.asarray(bq, np.float32)
    bk = np.asarray(bk, np.float32)
    bv = np.asarray(bv, np.float32)

    def prep_x(x):
        # x[b] is [S, D]; device wants xT in SBUF layout [128(p within dt),
        # half, dt, 512(seq)] where source row index = dt*128 + p.
        xT = x.T.astype(BFNP)                       # [D, S]
        a = xT.reshape(NDT, P, 2, 512)              # [dt, p, half, s]
        return np.ascontiguousarray(a.transpose(1, 2, 0, 3))

    def prep_w(w):
        # w slice is [D, DG] -> [128, dt, DG], row = dt*128 + p
        a = w.astype(BFNP).reshape(NDT, P, DG)
        return np.ascontiguousarray(a.transpose(1, 0, 2))

    def prep_wo(w):
        # w slice is [DG, D] -> [128, hp, D], row = hp*128 + p
        a = w.astype(BFNP).reshape(NJT, P, D)
        return np.ascontiguousarray(a.transpose(1, 0, 2))

    qP = [prep_x(q[b]) for b in range(B)]
    kP = [prep_x(k[b]) for b in range(B)]
    vP = [prep_x(v[b]) for b in range(B)]
    bcm = make_bcmask()

    in_maps = []
    for c in range(NCORES):
        b, g = divmod(c, 2)
        sl = slice(g * DG, (g + 1) * DG)
        in_maps.append({
            "xqT": qP[b],
            "xkT": kP[b],
            "xvT": vP[b],
            "wq": prep_w(Wq[:, sl]),
            "wk": prep_w(Wk[:, sl]),
            "wv": prep_w(Wv[:, sl]),
            "bq": np.ascontiguousarray(bq[sl]).reshape(1, DG),
            "bk": np.ascontiguousarray(bk[sl]).reshape(1, DG),
            "bv": np.ascontiguousarray(bv[sl]).reshape(1, DG).astype(BFNP),
            "wo": prep_wo(Wo[sl, :]),
            "bcmask": bcm,
        })
    return in_maps


def combine_outputs(parts, bo):
    bo = np.asarray(bo, np.float32)
    out = np.empty((B, S, D), np.float32)
    for b in range(B):
        out[b] = np.maximum(parts[2 * b] + parts[2 * b + 1] + bo[None, :], 0.0)
    return out


def run(in_maps, trace=False, **kwargs):
    from concourse.bass_utils import run_bass_kernel_spmd
    nc = get_nc()
    return run_bass_kernel_spmd(nc, in_maps, list(range(NCORES)),
                                trace=trace, **kwargs)


def kernel(q, k, v, Wq, bq, Wk, bk, Wv, bv, Wo, bo):
    in_maps = make_in_maps(q, k, v, Wq, bq, Wk, bk, Wv, bv, Wo, bo)
    res = run(in_maps)
    parts = [res.results[c]["out"] for c in range(NCORES)]
    return combine_outputs(parts, bo)


# revision 35
# speedup vs baseline: 1.4930x; 1.0855x over previous
"""Multi-head attention (Keras-style, relu-activated dense projections)
for Trainium2, SPMD across 8 NeuronCores.

Problem (full shapes):
    B, S, D, H = 4, 1024, 1024, 16 ; DH = 64
    qp = relu(q @ Wq + bq); kp = relu(k @ Wk + bk); vp = relu(v @ Wv + bv)
    per head h: scores = qh @ kh^T / 8 ; attn = softmax(scores)
    out = relu(concat_h(attn @ vh) @ Wo + bo)

Sharding: core c = (batch b = c//2, head-group g = c%2). Each core computes
the 8 heads of group g for batch b end-to-end and produces the partial
output projection  attn_out_g @ Wo[g*512:(g+1)*512, :]  (no bias / relu).
Host sums the two partials per batch, adds bo, applies relu.

v2 redesign vs the 241us baseline (which ran the PE half-clocked most of
the kernel because attention serialized scores->exp->attnV per head pair):
  - all matmul operands bf16 (halves DMA bytes, enables FWL weight loads);
    PSUM accumulation stays f32.
  - emission software-pipelines the whole kernel into the ACT exp windows:
    while exp(iter i) runs, PE does the next jt's Q/K projection, V
    projection slices, attnV of earlier iters and the first output
    projection chunk. PE never idles >3.4us -> HAM stays warm.
  - softmax denominator: DVE tree-sum over key tiles, then TWO masked
    ones-matmuls broadcast Z_A/Z_B straight into a [128,512] psum tile
    (replaces onescol reduce + staging copies + K=33 broadcast matmul).
  - projection relu+bias moved off ACT onto DVE (tensor_scalar add+max),
    so ACT runs exp back-to-back.
  - DMA: few big rearranged transfers, weights issued from GpSimd queue,
    activations from Sync, ordered so the first score matmul can start
    ~6us in.
"""

import numpy as np
import ml_dtypes
from contextlib import ExitStack

import concourse.bass as bass
import concourse.mybir as mybir
import concourse.tile as tile
from concourse import bacc

B, S, D, H = 4, 1024, 1024, 16
DG = 512          # feature slice per core (8 heads)
HL = 8            # heads per core
DH = 64
P = 128
NCORES = 8
NJT = DG // P     # 4 feature tiles == head pairs
NST = S // P      # 8 sequence tiles
NDT = D // P      # 8 contraction tiles for projections
NPC = S // 512    # 2 query chunks of 512

F32 = mybir.dt.float32
BF16 = mybir.dt.bfloat16
AF = mybir.ActivationFunctionType
ALU = mybir.AluOpType
BFNP = ml_dtypes.bfloat16


def build_bass(has_vbias=False):
    nc = bacc.Bacc("TRN2", target_bir_lowering=False, debug=False,
                   num_devices=NCORES)

    # x/w tensors arrive host-pre-arranged in SBUF layout: [128(partition),
    # half, dt, 512] resp. [128, dt, dout] — every DMA line is one
    # contiguous per-partition run (128 descriptors, cheap issue).
    xqT = nc.dram_tensor("xqT", [P, 2, NDT, 512], BF16,
                         kind="ExternalInput").ap()
    xkT = nc.dram_tensor("xkT", [P, 2, NDT, 512], BF16,
                         kind="ExternalInput").ap()
    xvT = nc.dram_tensor("xvT", [P, 2, NDT, 512], BF16,
                         kind="ExternalInput").ap()
    wq = nc.dram_tensor("wq", [P, NDT, DG], BF16, kind="ExternalInput").ap()
    wk = nc.dram_tensor("wk", [P, NDT, DG], BF16, kind="ExternalInput").ap()
    wv = nc.dram_tensor("wv", [P, NDT, DG], BF16, kind="ExternalInput").ap()
    bq = nc.dram_tensor("bq", [P, NJT], F32, kind="ExternalInput").ap()
    bk = nc.dram_tensor("bk", [P, NJT], F32, kind="ExternalInput").ap()
    bv = nc.dram_tensor("bv", [1, DG], BF16, kind="ExternalInput").ap()
    wo = nc.dram_tensor("wo", [P, NJT, D], BF16, kind="ExternalInput").ap()
    bcm_in = nc.dram_tensor("bcmask", [P, 2 * P], BF16,
                            kind="ExternalInput").ap()
    out = nc.dram_tensor("out", [S, D], F32, kind="ExternalOutput").ap()

    with tile.TileContext(nc) as tc, ExitStack() as ctx, \
            nc.allow_low_precision(reason="bf16 compute is intentional"):
        consts = ctx.enter_context(tc.tile_pool(name="consts", bufs=1))
        xpool = ctx.enter_context(tc.tile_pool(name="xpool", bufs=5))
        wpool = ctx.enter_context(tc.tile_pool(name="wpool", bufs=3))
        wopool = ctx.enter_context(tc.tile_pool(name="wopool", bufs=1))
        qkpool = ctx.enter_context(tc.tile_pool(name="qkpool", bufs=1))
        vpool = ctx.enter_context(tc.tile_pool(name="vpool", bufs=1))
        epool = ctx.enter_context(tc.tile_pool(name="epool", bufs=4))
        aopool = ctx.enter_context(tc.tile_pool(name="aopool", bufs=1))
        t1pool = ctx.enter_context(tc.tile_pool(name="t1pool", bufs=1))
        espool = ctx.enter_context(tc.tile_pool(name="espool", bufs=2))
        rpool = ctx.enter_context(tc.tile_pool(name="rpool", bufs=4))
        outpool = ctx.enter_context(tc.tile_pool(name="outpool", bufs=2))

        psA = ctx.enter_context(tc.tile_pool(name="psA", bufs=2, space="PSUM"))
        psP = ctx.enter_context(tc.tile_pool(name="psP", bufs=2, space="PSUM"))
        psB = ctx.enter_context(tc.tile_pool(name="psB", bufs=1, space="PSUM"))
        psD = ctx.enter_context(tc.tile_pool(name="psD", bufs=1, space="PSUM"))

        # --- constants + weights, issued on the ACT HWDGE queue (GpSimd's
        # software DGE costs ~5us per DMA; SP+ACT are the HW DGE engines
        # and ACT is idle until the first exp).
        # weight tiles [128(din within dt), dt, dout]
        wkt = wpool.tile([P, NDT, DG], BF16, tag="w")
        nc.scalar.dma_start(out=wkt, in_=wk)
        def load_x(src, half):
            t = xpool.tile([P, NDT, 512], BF16, tag="x")
            nc.sync.dma_start(out=t, in_=src[:, half])
            return t

        # SP HWDGE ring is FIFO: order the x loads by first use.
        # xq half1 is loaded later (it reuses an xk slot freed after the
        # last k-projection; q-proj of the pc1 half is deferred to W4+).
        xk = [load_x(xkT, 0), None]
        xq = [load_x(xqT, 0), None]
        xk[1] = load_x(xkT, 1)
        xv = [load_x(xvT, 0), load_x(xvT, 1)]
        wqt = wpool.tile([P, NDT, DG], BF16, tag="w")
        nc.scalar.dma_start(out=wqt, in_=wq)
        bqT = consts.tile([P, NJT], F32, tag="bqT")
        nc.scalar.dma_start(out=bqT, in_=bq)
        bkT = consts.tile([P, NJT], F32, tag="bkT")
        nc.scalar.dma_start(out=bkT, in_=bk)
        bcm = consts.tile([P, 2 * P], BF16, tag="bcm")
        nc.scalar.dma_start(out=bcm, in_=bcm_in)
        wvt = wpool.tile([P, NDT, DG], BF16, tag="w")
        nc.scalar.dma_start(out=wvt, in_=wv)
        wo3 = wopool.tile([P, NJT, D], BF16, tag="wo3")
        nc.scalar.dma_start(out=wo3, in_=wo)
        if has_vbias:
            # broadcast bv across partitions for the DVE bias-add
            bvb = consts.tile([P, DG], BF16, tag="bvb")
            nc.scalar.dma_start(out=bvb, in_=bv.to_broadcast([P, DG]))

        # --- persistent activations ---------------------------------------
        qpT = qkpool.tile([P, NJT, S], BF16, tag="qpT")
        kpT = qkpool.tile([P, NJT, S], BF16, tag="kpT")
        vpa = vpool.tile([P, NST, DG], BF16, tag="vpa")
        aoT3 = aopool.tile([P, NJT, S], BF16, tag="aoT3")

        # ------------------------------------------------------------------
        def emit_qkproj(wt, xs, bT, dst, jt, half):
            """dst[:, jt, half*512:] = relu(w[:,jt-cols].T @ x[half] + b)"""
            ps = psP.tile([P, 512], F32, tag="pp")
            for dt_ in range(NDT):
                nc.tensor.matmul(
                    ps, lhsT=wt[:, dt_, jt * P:(jt + 1) * P],
                    rhs=xs[half][:, dt_, :],
                    start=(dt_ == 0), stop=(dt_ == NDT - 1))
            nc.vector.tensor_scalar(
                dst[:, jt, half * 512:(half + 1) * 512], ps,
                scalar1=bT[:, jt:jt + 1], scalar2=0.0,
                op0=ALU.add, op1=ALU.max)

        def emit_vproj(st):
            """vpa[:, st, :] = relu(x_v[st-cols].T @ wv + bv)"""
            ps = psP.tile([P, 512], F32, tag="pp")
            half, q = st // 4, st % 4
            for dt_ in range(NDT):
                nc.tensor.matmul(
                    ps, lhsT=xv[half][:, dt_, q * P:(q + 1) * P],
                    rhs=wvt[:, dt_, :],
                    start=(dt_ == 0), stop=(dt_ == NDT - 1))
            if has_vbias:
                nc.vector.tensor_add(ps, ps, bvb)
            nc.vector.tensor_scalar_max(vpa[:, st, :], ps, 0.0)

        def emit_scores_exp(pc, hp, ex=None, uts=range(NST)):
            """returns ex [128(k), ut, 1024] bf16 (head A cols 0:512, B 512:)"""
            if ex is None:
                ex = epool.tile([P, NST, 1024], BF16, tag="exp")
            pslice = slice(pc * 512, (pc + 1) * 512)
            for ut in uts:
                uslice = slice(ut * P, (ut + 1) * P)
                pw = psA.tile([P, 1024], F32, tag="ps")
                nc.tensor.matmul(
                    pw[:, 0:512],
                    lhsT=kpT[0:DH, hp, uslice], rhs=qpT[0:DH, hp, pslice],
                    start=True, stop=True)
                nc.tensor.matmul(
                    pw[:, 512:1024],
                    lhsT=kpT[DH:P, hp, uslice], rhs=qpT[DH:P, hp, pslice],
                    start=True, stop=True)
                nc.scalar.activation(ex[:, ut, :], pw, AF.Exp, scale=0.125)
            return ex

        def emit_finz(ex):
            """softmax denominators: rcp [128,512] f32, rows 0:64 = 1/Z_A
            broadcast, rows 64:128 = 1/Z_B. Partition reduction runs on the
            (otherwise idle) GpSimd engine, keeping it off the PE queue."""
            t1 = t1pool.tile([P, 4, 1024], BF16, tag="t1")
            nc.vector.tensor_add(t1, ex[:, 0:4, :], ex[:, 4:8, :])
            nc.vector.tensor_add(t1[:, 0:2, :], t1[:, 0:2, :], t1[:, 2:4, :])
            exsum = espool.tile([P, 1024], BF16, tag="exsum")
            nc.vector.tensor_add(exsum, t1[:, 0, :], t1[:, 1, :])
            # masked ones-matmuls reduce partitions AND broadcast Z in one
            # step: rows 0:64 <- Z_A, rows 64:128 <- Z_B. Shares psB's bank
            # with nt (both drain quickly into DVE).
            zps = psB.tile([P, 512], F32, tag="nt")
            nc.tensor.matmul(zps, lhsT=bcm[:, 0:P], rhs=exsum[:, 0:512],
                             start=True, stop=False)
            nc.tensor.matmul(zps, lhsT=bcm[:, P:2 * P], rhs=exsum[:, 512:1024],
                             start=False, stop=True)
            rcp = rpool.tile([P, 512], F32, tag="rcp")
            nc.vector.reciprocal_approx_fast(rcp, zps)
            return rcp

        def emit_attnv(pc, hp, ex, rcp):
            hA, hB = 2 * hp, 2 * hp + 1
            nt = psB.tile([P, 512], F32, tag="nt")
            for ut in range(NST):
                nc.tensor.matmul(
                    nt[0:DH, :],
                    lhsT=vpa[:, ut, hA * DH:(hA + 1) * DH],
                    rhs=ex[:, ut, 0:512],
                    start=(ut == 0), stop=(ut == NST - 1),
                    skip_group_check=True)
                nc.tensor.matmul(
                    nt[DH:P, :],
                    lhsT=vpa[:, ut, hB * DH:(hB + 1) * DH],
                    rhs=ex[:, ut, 512:1024],
                    start=(ut == 0), stop=(ut == NST - 1),
                    skip_group_check=True)
            nc.vector.tensor_mul(aoT3[:, hp, pc * 512:(pc + 1) * 512], nt, rcp)

        def emit_outproj(pt, copy_eng, alt_bank=False):
            os_ = outpool.tile([P, 1024], F32, tag="os")
            for jj in range(2):
                # in the tail the projections are done, so psP's banks are
                # free: alternate with psD so output groups double-buffer.
                pool = psP if (alt_bank and jj == 1) else psD
                po_ = pool.tile([P, 512], F32, tag="pp" if pool is psP
                                else "po")
                for hp in range(NJT):
                    nc.tensor.matmul(
                        po_, lhsT=aoT3[:, hp, pt * P:(pt + 1) * P],
                        rhs=wo3[:, hp, jj * 512:(jj + 1) * 512],
                        start=(hp == 0), stop=(hp == NJT - 1))
                if copy_eng == "scalar":
                    nc.scalar.copy(os_[:, jj * 512:(jj + 1) * 512], po_)
                else:
                    nc.vector.tensor_copy(os_[:, jj * 512:(jj + 1) * 512], po_)
            nc.sync.dma_start(out=out[pt * P:(pt + 1) * P, :], in_=os_)

        # --- software-pipelined emission ----------------------------------
        # Scores for pc-chunk 0 only need the pc0 half of qpT, so pc1-half
        # q-projections are deferred to W4+; each window's PE work is sized
        # to fit under one 8-exp ACT block (~9.2us).
        # W0: minimum work before the first exp can start — scores over the
        # first 4 key tiles only need the pc0 half of kpT jt0.
        emit_qkproj(wkt, xk, bkT, kpT, 0, 0)
        emit_qkproj(wqt, xq, bqT, qpT, 0, 0)
        ex00 = emit_scores_exp(0, 0, uts=range(0, 4))
        emit_qkproj(wkt, xk, bkT, kpT, 0, 1)
        emit_scores_exp(0, 0, ex=ex00, uts=range(4, NST))
        # W1 (under exp(0,0))
        emit_qkproj(wkt, xk, bkT, kpT, 1, 0)
        emit_qkproj(wkt, xk, bkT, kpT, 1, 1)
        emit_qkproj(wqt, xq, bqT, qpT, 1, 0)
        emit_vproj(0)
        emit_vproj(1)
        r00 = emit_finz(ex00)
        ex01 = emit_scores_exp(0, 1)
        # W2 (under exp(0,1))
        emit_qkproj(wkt, xk, bkT, kpT, 2, 0)
        emit_qkproj(wkt, xk, bkT, kpT, 2, 1)
        emit_qkproj(wqt, xq, bqT, qpT, 2, 0)
        emit_vproj(2)
        emit_vproj(3)
        r01 = emit_finz(ex01)
        ex02 = emit_scores_exp(0, 2)
        # W3 (under exp(0,2)): last k-proj frees two x slots -> xq half1.
        emit_qkproj(wkt, xk, bkT, kpT, 3, 0)
        emit_qkproj(wkt, xk, bkT, kpT, 3, 1)
        xq[1] = load_x(xqT, 1)
        emit_qkproj(wqt, xq, bqT, qpT, 3, 0)
        emit_vproj(4)
        emit_vproj(5)
        r02 = emit_finz(ex02)
        ex03 = emit_scores_exp(0, 3)
        # W4 (under exp(0,3)): finish V proj, first attnV, start pc1 q-proj.
        emit_vproj(6)
        emit_vproj(7)
        emit_qkproj(wqt, xq, bqT, qpT, 0, 1)
        emit_attnv(0, 0, ex00, r00)
        r03 = emit_finz(ex03)
        ex10 = emit_scores_exp(1, 0)
        # W5 (under exp(1,0))
        emit_attnv(0, 1, ex01, r01)
        emit_attnv(0, 2, ex02, r02)
        emit_qkproj(wqt, xq, bqT, qpT, 1, 1)
        r10 = emit_finz(ex10)
        ex11 = emit_scores_exp(1, 1)
        # W6 (under exp(1,1))
        emit_attnv(0, 3, ex03, r03)
        emit_qkproj(wqt, xq, bqT, qpT, 2, 1)
        emit_outproj(0, "vector")
        emit_outproj(1, "vector")
        r11 = emit_finz(ex11)
        ex12 = emit_scores_exp(1, 2)
        # W7 (under exp(1,2))
        emit_attnv(1, 0, ex10, r10)
        emit_qkproj(wqt, xq, bqT, qpT, 3, 1)
        emit_outproj(2, "vector")
        emit_outproj(3, "vector")
        r12 = emit_finz(ex12)
        ex13 = emit_scores_exp(1, 3)
        # W8 (under exp(1,3))
        emit_attnv(1, 1, ex11, r11)
        emit_attnv(1, 2, ex12, r12)
        r13 = emit_finz(ex13)
        # tail
        emit_attnv(1, 3, ex13, r13)
        for pt in range(4, 8):
            emit_outproj(pt, "scalar", alt_bank=True)

    nc.compile()
    return nc


_CACHE = {}


def get_nc(has_vbias=False):
    if has_vbias not in _CACHE:
        _CACHE[has_vbias] = build_bass(has_vbias)
    return _CACHE[has_vbias]


def make_bcmask():
    m = np.zeros((P, 2 * P), np.float32)
    m[:, 0:DH] = 1.0          # bcmA: out rows 0:64  <- Z_A
    m[:, P + DH:2 * P] = 1.0  # bcmB: out rows 64:128 <- Z_B
    return m.astype(BFNP)


def make_in_maps(q, k, v, Wq, bq, Wk, bk, Wv, bv, Wo, bo):
    q = np.asarray(q, np.float32)
    k = np.asarray(k, np.float32)
    v = np.asarray(v, np.float32)
    Wq = np.asarray(Wq, np.float32)
    Wk = np.asarray(Wk, np.float32)
    Wv = np.asarray(Wv, np.float32)
    Wo = np.asarray(Wo, np.float32)
    bq = np.asarray(bq, np.float32)
    bk = np.asarray(bk, np.float32)
    bv = np.asarray(bv, np.float32)

    def prep_x(x):
        # x[b] is [S, D]; device wants xT in SBUF layout [128(p within dt),
        # half, dt, 512(seq)] where source row index = dt*128 + p.
        xT = x.T.astype(BFNP)                       # [D, S]
        a = xT.reshape(NDT, P, 2, 512)              # [dt, p, half, s]
        return np.ascontiguousarray(a.transpose(1, 2, 0, 3))

    def prep_w(w):
        # w slice is [D, DG] -> [128, dt, DG], row = dt*128 + p
        a = w.astype(BFNP).reshape(NDT, P, DG)
        return np.ascontiguousarray(a.transpose(1, 0, 2))

    def prep_wo(w):
        # w slice is [DG, D] -> [128, hp, D], row = hp*128 + p
        a = w.astype(BFNP).reshape(NJT, P, D)
        return np.ascontiguousarray(a.transpose(1, 0, 2))

    def prep_b(b_):
        # [DG] -> [128, jt]: partition-major layout for the per-partition
        # DVE bias operand (tiny, but keeps the DMA contiguous).
        return np.ascontiguousarray(b_.reshape(NJT, P).T)

    qP = [prep_x(q[b]) for b in range(B)]
    kP = [prep_x(k[b]) for b in range(B)]
    vP = [prep_x(v[b]) for b in range(B)]
    bcm = make_bcmask()

    in_maps = []
    for c in range(NCORES):
        b, g = divmod(c, 2)
        sl = slice(g * DG, (g + 1) * DG)
        in_maps.append({
            "xqT": qP[b],
            "xkT": kP[b],
            "xvT": vP[b],
            "wq": prep_w(Wq[:, sl]),
            "wk": prep_w(Wk[:, sl]),
            "wv": prep_w(Wv[:, sl]),
            "bq": prep_b(bq[sl]),
            "bk": prep_b(bk[sl]),
            "bv": np.ascontiguousarray(bv[sl]).reshape(1, DG).astype(BFNP),
            "wo": prep_wo(Wo[sl, :]),
            "bcmask": bcm,
        })
    return in_maps


def combine_outputs(parts, bo):
    bo = np.asarray(bo, np.float32)
    out = np.empty((B, S, D), np.float32)
    for b in range(B):
        out[b] = np.maximum(parts[2 * b] + parts[2 * b + 1] + bo[None, :], 0.0)
    return out


def run(in_maps, trace=False, has_vbias=False, **kwargs):
    from concourse.bass_utils import run_bass_kernel_spmd
    nc = get_nc(has_vbias)
    return run_bass_kernel_spmd(nc, in_maps, list(range(NCORES)),
                                trace=trace, **kwargs)


def kernel(q, k, v, Wq, bq, Wk, bk, Wv, bv, Wo, bo):
    in_maps = make_in_maps(q, k, v, Wq, bq, Wk, bk, Wv, bv, Wo, bo)
    res = run(in_maps, has_vbias=bool(np.any(np.asarray(bv))))
    parts = [res.results[c]["out"] for c in range(NCORES)]
    return combine_outputs(parts, bo)
